# revision 34
# baseline (speedup 1.0000x reference)
import sys
from contextlib import ExitStack

import numpy as np

sys.path.insert(0, "/opt/trn_rl_repo")

import ml_dtypes
import concourse.bass as bass
import concourse.tile as tile
from concourse import bacc, mybir
from concourse._compat import with_exitstack
from concourse.bass_utils import run_bass_kernel_spmd

B, L, D, H, NS = 8, 1024, 1024, 16, 8
HD = D // H
LS = L - NS
NCORES = 8
NEG = -1.0e9
BF = mybir.dt.bfloat16
F32 = mybir.dt.float32
EXP = mybir.ActivationFunctionType.Exp
COPY = mybir.ActivationFunctionType.Copy
MULT = mybir.AluOpType.mult

_CACHE = {}
TRACE = False


@with_exitstack
def _attention_kernel(ctx: ExitStack, tc: tile.TileContext, aps: dict):
    nc = tc.nc

    sb = ctx.enter_context(tc.tile_pool(name="sb", bufs=1))
    ptp = ctx.enter_context(tc.tile_pool(name="ptp", bufs=4))
    stg = ctx.enter_context(tc.tile_pool(name="stg", bufs=2))
    ps = ctx.enter_context(tc.tile_pool(name="ps", bufs=1, space="PSUM"))
    dram = ctx.enter_context(tc.tile_pool(name="dram", bufs=1, space="DRAM"))

    xT = sb.tile([128, 8192], BF)
    wq = sb.tile([128, 8192], BF)
    wk = sb.tile([128, 8192], BF)
    wv = sb.tile([128, 8192], BF)
    qT = sb.tile([128, 8192], BF)
    kT = sb.tile([128, 8192], BF)
    vb = sb.tile([128, 8448], BF)
    oU = sb.tile([128, 16384], BF)
    oT = sb.tile([128, 8192], BF)
    nsb = sb.tile([8, 3072], BF)
    fullg = sb.tile([64, 3072], BF)
    xtl = sb.tile([128, 64], BF)
    sel = sb.tile([64, 8], BF)
    tri = sb.tile([128, 128], BF)
    idn = sb.tile([128, 128], BF)
    pat = sb.tile([64, 2048], BF)
    mb = sb.tile([128, 8], F32)
    sums_b = sb.tile([64, 512], BF)
    sums_f = sb.tile([64, 512], F32)
    rec_f = sb.tile([64, 512], F32)
    rec_b = sb.tile([64, 512], BF)

    dummy = sb.tile([128, 512], BF)
    nc.gpsimd.memset(dummy[:], 0.0)
    trash = ps.tile([128, 512], F32, tag="op", bufs=1)
    for _ in range(14):
        nc.tensor.matmul(trash[:], dummy[:, 0:128], dummy[:], start=True, stop=True)

    nc.scalar.dma_start(xtl[:], aps["xtails"][:])
    for dt in range(8):
        cs = slice(dt * 1024, (dt + 1) * 1024)
        nc.scalar.dma_start(xT[:, cs], aps["xT"][:, cs])
        nc.sync.dma_start(wq[:, cs], aps["wq"][:, cs])

    cnt = 0

    def acc_tile(tags=("acc",), bufs=(2,)):
        nonlocal cnt
        i = cnt % len(tags)
        t = ps.tile([128, 512], F32, name="acc", tag=tags[i], bufs=bufs[i])
        cnt += 1
        return t

    def qk_proj(w, dst, et, lg):
        acc = acc_tile()
        for dt in range(8):
            nc.tensor.matmul(
                acc[:],
                w[:, dt * 1024 + et * 128: dt * 1024 + et * 128 + 128],
                xT[:, dt * 1024 + lg * 512: dt * 1024 + lg * 512 + 512],
                start=(dt == 0),
                stop=(dt == 7),
            )
        nc.vector.tensor_copy(
            dst[:, et * 1024 + lg * 512: et * 1024 + lg * 512 + 512], acc[:]
        )

    def v_proj(lt, eg):
        acc = acc_tile()
        for dt in range(8):
            nc.tensor.matmul(
                acc[:],
                xT[:, dt * 1024 + lt * 128: dt * 1024 + lt * 128 + 128],
                wv[:, dt * 1024 + eg * 512: dt * 1024 + eg * 512 + 512],
                start=(dt == 0),
                stop=(dt == 7),
            )
        src3 = acc.rearrange("p (h c) -> p h c", h=8)
        base = lt * 1056 + eg * 528
        dst3 = vb[:, base:base + 528].rearrange("p (h c) -> p h c", h=8)[:, :, 0:64]
        nc.vector.tensor_copy(dst3, src3)

    for dt in range(8):
        cs = slice(dt * 1024, (dt + 1) * 1024)
        nc.sync.dma_start(wk[:, cs], aps["wk"][:, cs])
        nc.scalar.dma_start(wv[:, cs], aps["wv"][:, cs])
    nc.scalar.dma_start(tri[:], aps["tri"][:])
    nc.scalar.dma_start(idn[:], aps["ident"][:])
    nc.scalar.dma_start(mb[:], aps["maskbias"][:])
    nc.scalar.dma_start(sel[:], aps["sel"][:])
    nc.scalar.dma_start(pat[:], aps["pat"][:])

    for et in range(8):
        qk_proj(wq, qT, et, 0)
    for et in range(8):
        qk_proj(wk, kT, et, 0)

    pp01 = ps.tile([8, 1024], F32, tag="sp", bufs=2)
    pp23 = ps.tile([8, 1024], F32, tag="sp", bufs=2)
    pp45 = ps.tile([8, 1024], F32, tag="op", bufs=1)
    ns_dst = [
        pp01[:, 0:512], pp01[:, 512:1024],
        pp23[:, 0:512], pp23[:, 512:1024],
        pp45[:, 0:512], pp45[:, 512:1024],
    ]
    with tc.tile_pool(name="wnsp", bufs=2) as wnsp:
        for dt in range(8):
            for hf in range(2):
                wt = wnsp.tile([128, 1536], BF, name="wt")
                c0 = dt * 3072 + hf * 1536
                eng = nc.sync if hf == 0 else nc.scalar
                eng.dma_start(wt[:], aps["wns"][:, c0:c0 + 1536])
                for ck3 in range(3):
                    ck = hf * 3 + ck3
                    nc.tensor.matmul(
                        ns_dst[ck],
                        xtl[:, dt * 8:dt * 8 + 8],
                        wt[:, ck3 * 512:(ck3 + 1) * 512],
                        start=(dt == 0),
                        stop=(dt == 7),
                    )
    nc.vector.tensor_copy(nsb[:, 0:1024], pp01[0:8, :])
    nc.vector.tensor_copy(nsb[:, 1024:2048], pp23[0:8, :])
    nc.vector.tensor_copy(nsb[:, 2048:3072], pp45[0:8, :])

    gin = dram.tile([8, 3072], BF)
    gout = dram.tile([64, 3072], BF)
    nc.gpsimd.dma_start(gin[:], nsb[:])
    nc.gpsimd.collective_compute(
        "AllGather",
        mybir.AluOpType.bypass,
        replica_groups=[list(range(NCORES))],
        ins=[gin.opt()],
        outs=[gout.opt()],
    )
    nc.gpsimd.dma_start(fullg[:], gout[:])


    for lt in range(8):
        blk = vb[:, lt * 1056:(lt + 1) * 1056]
        nc.vector.memset(
            blk.rearrange("p (h c) -> p h c", h=16)[:, :, 64:65], 1.0
        )

    for lt in range(4):
        v_proj(lt, 0)
        v_proj(lt, 1)

    def unit(et, g):
        nj = 4 + 4 * g
        qcol = et * 1024 + g * 512
        su = (et + 8 * g) * 1024
        op = ps.tile([128, 1024], F32, name="op", tag="op", bufs=1)

        sps = {}

        def sc(j):
            sp = ps.tile([128, 1024], F32, name="sp", tag="sp", bufs=2)
            diag = j * 128 >= g * 512
            ld = max(0, j * 128 - g * 512)
            for par in range(2):
                r0 = par * 64
                nc.tensor.matmul(
                    sp[:, par * 512 + ld:(par + 1) * 512],
                    kT[r0:r0 + 64, et * 1024 + j * 128: et * 1024 + (j + 1) * 128],
                    qT[r0:r0 + 64, qcol + ld:qcol + 512],
                    start=True,
                    stop=(not diag),
                )
            if diag:
                for par in range(2):
                    nc.tensor.matmul(
                        sp[:, par * 512 + ld: par * 512 + ld + 128],
                        idn[:],
                        tri[:],
                        start=False,
                        stop=True,
                    )
            sps[j] = (sp, ld)

        sc(0)
        if nj > 1:
            sc(1)
        for j in range(nj):
            sp, ld = sps.pop(j)
            pt = ptp.tile([128, 1024], BF, name="pt")
            src = sp.rearrange("p (h q) -> p h q", h=2)[:, :, ld:512]
            dst = pt.rearrange("p (h q) -> p h q", h=2)[:, :, ld:512]
            nc.scalar.activation(dst, src, EXP, bias=mb[:, j:j + 1], scale=0.125)
            if j + 2 < nj:
                sc(j + 2)
            for par in range(2):
                h = 2 * et + par
                nc.tensor.matmul(
                    op[0:65, par * 512 + ld:(par + 1) * 512],
                    vb[:, j * 1056 + h * 66: j * 1056 + h * 66 + 65],
                    pt[:, par * 512 + ld: (par + 1) * 512],
                    start=(j == 0),
                    stop=(j == nj - 1),
                )
        nc.vector.tensor_copy(oU[0:65, su:su + 1024], op[0:65, :])

    def norm_part(g):
        strip = 32 * g
        s0 = 8 * g * 1024
        src_ap = oU[64:65, s0:s0 + 8192].rearrange("p (a b) -> p a b", a=16)
        nc.sync.dma_start(sums_b[strip:strip + 16, :], src_ap)
        nc.vector.tensor_copy(sums_f[strip:strip + 16, :], sums_b[strip:strip + 16, :])
        nc.vector.reciprocal(rec_f[strip:strip + 16, :], sums_f[strip:strip + 16, :])
        nc.vector.tensor_copy(rec_b[strip:strip + 16, :], rec_f[strip:strip + 16, :])
        for et in range(8):
            w = 2 * et + g
            su = (et + 8 * g) * 1024
            rr = slice(strip, strip + 16)
            bbp = ps.tile([128, 512], F32, name="bbp", tag="acc", bufs=2)
            nc.tensor.matmul(
                bbp[0:64, :],
                pat[rr, w * 128: w * 128 + 64],
                rec_b[rr, :],
                start=True, stop=True,
            )
            nc.tensor.matmul(
                bbp[64:128, :],
                pat[rr, w * 128 + 64: w * 128 + 128],
                rec_b[rr, :],
                start=True, stop=True,
            )
            ow = slice(w * 512, (w + 1) * 512)
            nc.vector.tensor_tensor(
                oT[0:64, ow], oU[0:64, su:su + 512], bbp[0:64, :], MULT
            )
            nc.vector.tensor_tensor(
                oT[64:128, ow], oU[0:64, su + 512:su + 1024], bbp[64:128, :], MULT
            )

    for et in range(8):
        unit(et, 0)
        qk_proj(wq, qT, et, 1)
        qk_proj(wk, kT, et, 1)
        if et < 4:
            v_proj(4 + et, 0)
            v_proj(4 + et, 1)

    wo = sb.tile([128, 8192], BF, tag="wq")
    nc.sync.dma_start(wo[:], aps["wo"][:])

    def outproj(lt, eg, cp_engine):
        acc = acc_tile(tags=("acc", "sp"), bufs=(2, 2))
        for et in range(8):
            nc.tensor.matmul(
                acc[:],
                oT[:, et * 1024 + lt * 128: et * 1024 + lt * 128 + 128],
                wo[:, et * 1024 + eg * 512: et * 1024 + eg * 512 + 512],
                start=(et == 0),
                stop=(et == 7),
            )
        ys = stg.tile([128, 512], F32, name="ys")
        if cp_engine == "v":
            nc.vector.tensor_copy(ys[:], acc[:])
        else:
            nc.scalar.activation(ys[:], acc[:], COPY, scale=1.0)
        nc.gpsimd.dma_start(
            aps["y"][lt * 128:(lt + 1) * 128, eg * 512:(eg + 1) * 512], ys[:]
        )

    for c2 in range(16):
        tp = ps.tile([128, 512], F32, name="tp", tag="acc", bufs=2)
        nc.tensor.matmul(
            tp[:, 0:8],
            fullg[:, c2 * 128:(c2 + 1) * 128],
            sel[:],
            start=True,
            stop=True,
        )
        dst = qT if c2 < 8 else kT
        et = c2 % 8
        nc.vector.tensor_copy(
            dst[:, et * 1024 + 1016: et * 1024 + 1024], tp[:, 0:8]
        )
    for vg in range(2):
        tpv = ps.tile([128, 512], F32, name="tpv", tag="acc", bufs=2)
        nc.tensor.matmul(
            tpv[0:8, :],
            sel[:],
            fullg[:, 2048 + vg * 512: 2048 + (vg + 1) * 512],
            start=True,
            stop=True,
        )
        vt = stg.tile([8, 512], BF, name="vt")
        nc.vector.tensor_copy(vt[:], tpv[0:8, :])
        src3 = vt.rearrange("p (h c) -> p h c", h=8)
        base = 7 * 1056 + vg * 528
        dst3 = vb[120:128, base:base + 528].rearrange("p (h c) -> p h c", h=8)[:, :, 0:64]
        nc.sync.dma_start(dst3, src3)

    for et in range(8):
        unit(et, 1)
        if et == 1:
            norm_part(0)
        if 4 <= et <= 5:
            outproj(et - 4, 0, "v")
            outproj(et - 4, 1, "v")
    outproj(2, 0, "v")
    outproj(2, 1, "v")
    norm_part(1)
    outproj(3, 0, "v")
    outproj(3, 1, "v")
    for lt in range(4, 8):
        outproj(lt, 0, "s")
        outproj(lt, 1, "v")


def _build():
    if "nc" in _CACHE:
        return _CACHE["nc"]
    nc = bacc.Bacc("TRN2", target_bir_lowering=False, debug=False, num_devices=NCORES)
    aps = {}
    for name, shape, dt in [
        ("xT", [128, 8192], BF),
        ("wq", [128, 8192], BF),
        ("wk", [128, 8192], BF),
        ("wv", [128, 8192], BF),
        ("wo", [128, 8192], BF),
        ("wns", [128, 24576], BF),
        ("xtails", [128, 64], BF),
        ("sel", [64, 8], BF),
        ("tri", [128, 128], BF),
        ("ident", [128, 128], BF),
        ("pat", [64, 2048], BF),
        ("maskbias", [128, 8], F32),
    ]:
        aps[name] = nc.dram_tensor(name, shape, dt, kind="ExternalInput").ap()
    aps["y"] = nc.dram_tensor("y", [1024, 1024], F32, kind="ExternalOutput").ap()

    with tile.TileContext(nc) as tc:
        _attention_kernel(tc, aps)
    nc.compile()
    _CACHE["nc"] = nc
    return nc


def _pack8(a):
    r, c = a.shape
    return np.ascontiguousarray(
        a.reshape(8, 128, c).transpose(1, 0, 2).reshape(128, 8 * c)
    )


def kernel(x, key_padding_mask, Wq_s, Wk_s, Wv_s, Wq_ns, Wk_ns, Wv_ns, W_out, **kw):
    x = np.asarray(x, np.float32)
    mask = np.asarray(key_padding_mask)
    bf = ml_dtypes.bfloat16

    wq_h = _pack8(np.asarray(Wq_s, np.float32).T.astype(bf))
    wk_h = _pack8(np.asarray(Wk_s, np.float32).T.astype(bf))
    wv_h = _pack8(np.asarray(Wv_s, np.float32).T.astype(bf))
    wo_h = _pack8(np.asarray(W_out, np.float32).T.astype(bf))
    tri_h = np.where(
        np.arange(128)[:, None] <= np.arange(128)[None, :], 0.0, NEG
    ).astype(bf)
    ident_h = np.eye(128, dtype=np.float32).astype(bf)
    pat_h = np.zeros((64, 2048), bf)
    for et in range(8):
        for g in range(2):
            w = 2 * et + g
            for par in range(2):
                row = 32 * g + 2 * et + par
                pat_h[row, w * 128 + par * 64: w * 128 + par * 64 + 64] = 1.0

    Wq_ns = np.asarray(Wq_ns, np.float32)
    Wk_ns = np.asarray(Wk_ns, np.float32)
    Wv_ns = np.asarray(Wv_ns, np.float32)

    in_maps = []
    for c in range(NCORES):
        xT_h = _pack8(x[c].T.astype(bf))
        xtl_h = _pack8(x[:, LS + c, :].T.astype(bf))
        wns_h = _pack8(
            np.concatenate(
                [Wq_ns[c].T, Wk_ns[c].T, Wv_ns[c].T], axis=1
            ).astype(bf)
        )
        selm = np.zeros((64, 8), bf)
        for n in range(NS):
            selm[n * 8 + c, n] = 1.0
        mb_h = np.where(mask[c], 0.0, NEG).astype(np.float32).reshape(8, 128).T
        mb_h = np.ascontiguousarray(mb_h)
        in_maps.append(
            {
                "xT": xT_h,
                "wq": wq_h,
                "wk": wk_h,
                "wv": wv_h,
                "wo": wo_h,
                "wns": wns_h,
                "xtails": xtl_h,
                "sel": selm,
                "tri": tri_h,
                "ident": ident_h,
                "pat": pat_h,
                "maskbias": mb_h,
            }
        )

    nc = _build()
    res = run_bass_kernel_spmd(nc, in_maps, list(range(NCORES)), trace=TRACE)
    _CACHE["exec_time_ns"] = res.exec_time_ns
    _CACHE["res"] = res
    out = np.stack([res.results[c]["y"] for c in range(NCORES)], axis=0)
    return out.astype(np.float32)
